# revision 1
# baseline (speedup 1.0000x reference)
"""Trainium2 Bass kernel for the LeViT-style attention block.

Contract: kernel(**inputs) takes the FULL unsharded inputs (numpy) and
returns the FULL [128, 196, 576] float32 output. Internally shards the
batch dim across 8 NeuronCores (16 batches per core) and runs a single
SPMD Bass/Tile program via run_bass_kernel_spmd.

Math (per batch b):
  xn   = LayerNorm(x[b]) * g + beta                     [196, 576]
  qkv  = xn @ qkv_w.T + qkv_b      -> q,k,v per head
  S_h  = (q_h * kd^-0.5) @ k_h.T + bias_h               [196, 196]
  P_h  = softmax(S_h, axis=-1)
  O_h  = P_h @ v_h                                      [196, 128]
  out  = concat_h(O_h) @ proj_w.T + proj_b              [196, 576]

Implementation notes:
  - LN affine (g, beta) and the qk scale are folded into the QKV weights
    host-side; the device LN computes (x - mu) * rsqrt(var+eps) only.
  - All GEMMs run in float32r (PE streams 1 row/cycle vs 4 for fp32 when
    the moving free dim is >= 256; measured matmul rel-err ~1.5e-4).
    To hit the >=256 threshold, TWO batches are processed per iteration
    ("superbatch"), packing their 196 tokens side by side into a 392-wide
    free dim everywhere.
  - Activations live in channel-major ("transposed") layout on chip:
    Q.T/K.T [heads*32, 392] so per-head score matmuls need no transposes,
    and scores are computed as S.T = K Q.T in [key, query] layout so the
    P@V matmul is layout-natural (V is produced in token layout).
    Score/PV matmuls stream both batches' 392 columns and simply ignore
    the cross-batch half of the output (still 2x faster than fp32).
  - softmax: exp without max subtraction (scores are O(5); fp32-safe);
    denominator = ones-column matmul over keys on PE; reciprocal on DVE;
    broadcast back over partitions with a rank-1 ones-row matmul.
  - proj accumulates all 18 heads into 5 resident PSUM banks; the result
    is transposed back to token layout and DMA'd out.
"""

import os

os.environ.setdefault("MYCRO_LOCAL_CACHE", "1")

from contextlib import ExitStack

import numpy as np
import ml_dtypes

import concourse.bass as bass
import concourse.mybir as mybir
import concourse.tile as tile
from concourse import masks
from concourse.bass_utils import run_bass_kernel_spmd

# Problem shape (hardcoded per contest contract).
B, N, C = 128, 196, 576
H, KD, DV = 18, 32, 128
DH = H * DV            # 2304
LN_EPS = 1e-5
SCALE = KD ** -0.5
NCORES = 8
BPC = B // NCORES      # 16 batches per core
SB = 2                 # batches per "superbatch" iteration
NSB = BPC // SB        # 8
W = SB * N             # 392: packed two-batch free dim

FP32 = mybir.dt.float32
F32R = mybir.dt.float32r
BF16 = mybir.dt.bfloat16

# token-dim chunks (196 = 128 + 68)
TOK_CHUNKS = [(0, 128), (128, 68)]
# C-dim chunks (576 = 4*128 + 64)
C_CHUNKS = [(i * 128, min(128, C - i * 128)) for i in range((C + 127) // 128)]
NCC = len(C_CHUNKS)    # 5 contraction chunks / proj out-chunks
# Q.T/K.T m-chunks: 6 groups of 3 heads (96 rows) so per-head partition
# offsets stay in {0, 32, 64} (PE base-partition constraint).
NG = 6
GROWS = 3 * KD         # 96
# V free-dim chunks of 512 = 4 heads (2304 = 4*512 + 256); head-group g
# covers heads 4g..4g+3 (last group: 16,17).
V_CHUNKS = [(i * 512, min(512, DH - i * 512)) for i in range((DH + 511) // 512)]


def _split_multiwaits(nc):
    """This container's walrus rejects >1 sync-wait per instruction
    (TPB EVENTS struct has a single wait slot). Split extras into
    preceding same-engine NOPs — semantically identical."""
    for f in nc.m.functions:
        for blk in f.blocks:
            newlist = []
            changed = False
            for inst in blk.instructions:
                si = inst.sync_info
                waits = list(si.on_wait) if si is not None else []
                if len(waits) > 1:
                    changed = True
                    for j, w in enumerate(waits[:-1]):
                        nop = mybir.InstNoOp(name=f"{inst.name}_sw{j}", ins=[], outs=[])
                        nop.engine = inst.engine
                        nop.sync_info = mybir.SyncInfo(on_wait=[w], on_update=[])
                        newlist.append(nop)
                    inst.sync_info = mybir.SyncInfo(
                        on_wait=[waits[-1]], on_update=list(si.on_update)
                    )
                newlist.append(inst)
            if changed:
                blk.instructions = newlist


def _emit(ctx: ExitStack, tc: tile.TileContext, aps: dict, has_vbias: bool):
    nc = tc.nc
    x_d = aps["x"]          # [BPC, 196, 576] f32
    out_d = aps["out"]      # [BPC, 196, 576] f32
    wqk_d = aps["wqkt"]     # [576, 1152] f32r  (cols 0:576 Wq.T, 576:1152 Wk.T)
    wv_d = aps["wvt"]       # [576, 2304] f32r  (Wv.T, head-major columns)
    pw_d = aps["pwt"]       # [2304, 576] f32r  (proj_w.T, head-major rows)
    bias_d = aps["biasT"]   # [18, 196, 196] bf16 (per-head bias, [key, query])
    qkb_d = aps["qkb"]      # [128, 12] f32 (per-group q/k bias columns)
    pb_d = aps["pb"]        # [128, 5] f32 (proj bias chunks)
    vb_d = aps.get("vb")    # [128, 2304] f32 (replicated v bias) — optional

    cpool = ctx.enter_context(tc.tile_pool(name="consts", bufs=1))
    xpool = ctx.enter_context(tc.tile_pool(name="x", bufs=4))
    stat = ctx.enter_context(tc.tile_pool(name="stat", bufs=2))
    xnt_pool = ctx.enter_context(tc.tile_pool(name="xnt", bufs=1))
    qkt_pool = ctx.enter_context(tc.tile_pool(name="qkt", bufs=1))
    vpool = ctx.enter_context(tc.tile_pool(name="v", bufs=1))
    epool = ctx.enter_context(tc.tile_pool(name="e", bufs=3))
    rcpool = ctx.enter_context(tc.tile_pool(name="rc", bufs=2))
    rbpool = ctx.enter_context(tc.tile_pool(name="rb", bufs=1))
    onpool = ctx.enter_context(tc.tile_pool(name="on", bufs=2))
    otspool = ctx.enter_context(tc.tile_pool(name="ots", bufs=6))
    ftpool = ctx.enter_context(tc.tile_pool(name="ft", bufs=1))
    fpool = ctx.enter_context(tc.tile_pool(name="f", bufs=2))
    dram = ctx.enter_context(tc.tile_pool(name="dram", bufs=2, space="DRAM"))
    ps = ctx.enter_context(tc.tile_pool(name="ps", bufs=8, space="PSUM"))

    # ---- small on-chip constants first (PE transposes need ident; the
    # Pool engine must not queue these behind the bulk weight DMAs) ----
    ident = cpool.tile([128, 128], FP32, tag="ident", name="ident")
    masks.make_identity(nc, ident[:])
    ones_f = cpool.tile([128, 1], FP32, tag="ones_f", name="ones_f")
    nc.gpsimd.memset(ones_f[:], 1.0)
    onescol = cpool.tile([128, 1], F32R, tag="onescol", name="onescol")
    nc.vector.tensor_copy(onescol[:], ones_f[:])
    onesrow = cpool.tile([1, 128], F32R, tag="onesrow", name="onesrow")
    onesrow_f = cpool.tile([1, 128], FP32, tag="onesrow_f", name="onesrow_f")
    nc.gpsimd.memset(onesrow_f[:], 1.0)
    nc.vector.tensor_copy(onesrow[:], onesrow_f[:])
    epsb = cpool.tile([128, 1], FP32, tag="epsb", name="epsb")
    nc.gpsimd.memset(epsb[:], LN_EPS)
    ident_b = cpool.tile([128, 128], BF16, tag="ident_b", name="ident_b")
    masks.make_identity(nc, ident_b[:])

    # ---- constants (loaded once, resident) ----
    # Ordered by first use: Q/K weights, attention bias, V weights (by
    # column-group, just-in-time for the interleaved V GEMMs), then proj
    # weights. All on the Pool/SWDGE queue so x/out streaming (HWDGE) is
    # unaffected.
    wqk = []
    for ci, (c0, cs) in enumerate(C_CHUNKS):
        t = cpool.tile([128, 2 * C], F32R, tag=f"wqk{ci}", name=f"wqk{ci}")
        nc.gpsimd.dma_start(t[:cs, :], wqk_d[c0 : c0 + cs, :])
        wqk.append(t)
    qkb = cpool.tile([128, 2 * NG], FP32, tag="qkb", name="qkb")
    nc.gpsimd.dma_start(qkb[:], qkb_d[:])
    biasT = []
    for h in range(H):
        biasT.append(
            cpool.tile([128, 2 * N], BF16, tag=f"bias{h}", name=f"bias{h}")
        )
    wv = []
    for ci, (c0, cs) in enumerate(C_CHUNKS):
        wv.append(cpool.tile([128, DH], F32R, tag=f"wv{ci}", name=f"wv{ci}"))

    def load_bias(h):
        nc.gpsimd.dma_start(biasT[h][:128, 0:N], bias_d[h, 0:128, :])
        nc.gpsimd.dma_start(biasT[h][:68, N : 2 * N], bias_d[h, 128:196, :])

    # interleave V-weight column groups with the bias heads they feed, in
    # first-use order, so the first superbatch starts as early as possible
    for g, (n0, ns) in enumerate(V_CHUNKS):
        for ci, (c0, cs) in enumerate(C_CHUNKS):
            nc.gpsimd.dma_start(
                wv[ci][:cs, n0 : n0 + ns], wv_d[c0 : c0 + cs, n0 : n0 + ns]
            )
        for h in range(4 * g, min(4 * g + 4, H)):
            load_bias(h)
    pb = cpool.tile([128, NCC], FP32, tag="pb", name="pb")
    nc.gpsimd.dma_start(pb[:], pb_d[:])
    vb = None
    if has_vbias:
        vb = cpool.tile([128, DH], FP32, tag="vb", name="vb")
        nc.gpsimd.dma_start(vb[:], vb_d[:])
    pw = []
    for h in range(H):
        t = cpool.tile([128, C], F32R, tag=f"pw{h}", name=f"pw{h}")
        nc.gpsimd.dma_start(t[:], pw_d[h * DV : (h + 1) * DV, :])
        pw.append(t)

    inv_c = 1.0 / C
    EW = 848  # e-tile width: 4 blocks of 196 at 196-stride + 64 pad so every
              # 256-wide PV window starts at its block (over-reads are garbage
              # that lands in ignored output columns)

    def e_pair(e, tj, ts_):
        """Both batches' E for key-chunk tj: cols {tj*196} u {392+tj*196},
        as a [ts_, 2, 196] AP (free size 392)."""
        return e[:ts_, 0 : 4 * N].rearrange("p (q c n) -> p c q n", q=2, c=2, n=N)[
            :, tj
        ]

    lnstate = {}
    xstate = {}

    def ln_load(sbx, q, tj):
        b = sbx * SB + q
        t0, ts_ = TOK_CHUNKS[tj]
        xt = xpool.tile([128, C], FP32, tag="xb", name=f"xb{q}{tj}_{sbx}")
        nc.sync.dma_start(xt[:ts_, :], x_d[b, t0 : t0 + ts_, :])
        xstate[(sbx, q, tj)] = xt

    def ln_chunk(sbx, q, tj):
        """LayerNorm one loaded (batch, token-chunk) of x in place."""
        t0, ts_ = TOK_CHUNKS[tj]
        xt = xstate.pop((sbx, q, tj))
        negmu = stat.tile([128, 1], FP32, tag="negmu", name=f"nm{q}{tj}_{sbx}")
        nc.vector.tensor_reduce(
            negmu[:ts_], xt[:ts_, :], axis=mybir.AxisListType.X,
            op=mybir.AluOpType.add, negate=True,
        )
        nc.vector.tensor_scalar_mul(negmu[:ts_], negmu[:ts_], inv_c)
        # squared deviations in two halves (PSUM scratch; only the per-row
        # accumulators matter, the second half overwrites the first)
        sqp = ps.tile([128, 512], FP32, tag="ps", name=f"sqp{q}{tj}_{sbx}")
        ha = stat.tile([128, 1], FP32, tag="ha", name=f"ha{q}{tj}_{sbx}")
        hb = stat.tile([128, 1], FP32, tag="hb", name=f"hb{q}{tj}_{sbx}")
        nc.scalar.activation(
            sqp[:ts_, 0:288], xt[:ts_, 0:288],
            mybir.ActivationFunctionType.Square, bias=negmu[:ts_], accum_out=ha[:ts_],
        )
        nc.scalar.activation(
            sqp[:ts_, 0:288], xt[:ts_, 288:576],
            mybir.ActivationFunctionType.Square, bias=negmu[:ts_], accum_out=hb[:ts_],
        )
        ssq = stat.tile([128, 1], FP32, tag="ssq", name=f"ssq{q}{tj}_{sbx}")
        nc.vector.tensor_add(ssq[:ts_], ha[:ts_], hb[:ts_])
        std = stat.tile([128, 1], FP32, tag="std", name=f"std{q}{tj}_{sbx}")
        nc.scalar.activation(
            std[:ts_], ssq[:ts_], mybir.ActivationFunctionType.Sqrt,
            bias=epsb[:ts_], scale=inv_c,
        )
        r = stat.tile([128, 1], FP32, tag="r", name=f"r{q}{tj}_{sbx}")
        nc.vector.reciprocal(r[:ts_], std[:ts_])
        negmur = stat.tile([128, 1], FP32, tag="negmur", name=f"nr{q}{tj}_{sbx}")
        nc.vector.tensor_mul(negmur[:ts_], negmu[:ts_], r[:ts_])
        # xn0 = (x - mu) * r, in place
        nc.scalar.activation(
            xt[:ts_, :], xt[:ts_, :], mybir.ActivationFunctionType.Identity,
            bias=negmur[:ts_], scale=r[:ts_],
        )
        lnstate[(sbx, q, tj)] = xt

    pending_finals = []

    for q in range(SB):
        for tj in range(len(TOK_CHUNKS)):
            ln_load(0, q, tj)
            ln_chunk(0, q, tj)

    for sb in range(NSB):
        # ---- transpose xn -> xnT (channel-major, both batches packed) ----
        xnt = [
            xnt_pool.tile([128, W], F32R, tag=f"xnt{ci}", name=f"xnt{ci}_{sb}")
            for ci in range(NCC)
        ]
        for q in range(SB):
            for tj, (t0, ts_) in enumerate(TOK_CHUNKS):
                xt = lnstate.pop((sb, q, tj))
                for ci, (c0, cs) in enumerate(C_CHUNKS):
                    pt = ps.tile([128, 512], FP32, tag="ps", name=f"pst{q}{tj}{ci}_{sb}")
                    nc.tensor.transpose(
                        pt[:cs, :ts_], xt[:ts_, c0 : c0 + cs], ident[:ts_, :ts_]
                    )
                    col = q * N + t0
                    if ci % 2 == 0:
                        nc.scalar.copy(xnt[ci][:cs, col : col + ts_], pt[:cs, :ts_])
                    else:
                        nc.vector.tensor_copy(xnt[ci][:cs, col : col + ts_], pt[:cs, :ts_])

        if sb + 1 < NSB:
            # next superbatch's x loads: slots just freed by the transposes
            for q in range(SB):
                for tj in range(len(TOK_CHUNKS)):
                    ln_load(sb + 1, q, tj)

        # ---- Q.T / K.T GEMMs (12 groups of 96 rows, both batches wide) ----
        qkt = []  # 6 Q.T groups then 6 K.T groups, [96, 392] f32r
        for j in range(2 * NG):
            col0 = (j // NG) * C + (j % NG) * GROWS
            pq = ps.tile([128, 512], FP32, tag="ps", name=f"psqk{j}_{sb}")
            for ci, (c0, cs) in enumerate(C_CHUNKS):
                nc.tensor.matmul(
                    pq[:GROWS, :W], wqk[ci][:cs, col0 : col0 + GROWS],
                    xnt[ci][:cs, :W],
                    start=(ci == 0), stop=(ci == NCC - 1),
                )
            t = qkt_pool.tile([GROWS, W], F32R, tag=f"qkt{j}", name=f"qkt{j}_{sb}")
            nc.vector.tensor_scalar_add(t[:, :], pq[:GROWS, :W], qkb[:GROWS, j : j + 1])
            qkt.append(t)

        # ---- attention: software-pipelined head loop (skew 2) ----
        # Per-head O.T goes to a DRAM scratch; proj streams it back after the
        # loop. This keeps PSUM free for deep cross-head overlap.
        od = dram.tile([H, DV, W], F32R, tag="od", name=f"od{sb}")
        vgroups = {}
        estate = {}
        bstate = {}

        def stage_v(g, sb=sb, xnt=xnt, vgroups=vgroups):
            n0, ns = V_CHUNKS[g]
            vt = {}
            for q in range(SB):
                for tj, (t0, ts_) in enumerate(TOK_CHUNKS):
                    v = vpool.tile([128, 512], F32R, tag=f"v{q}{tj}",
                                   name=f"v{q}{tj}g{g}_{sb}")
                    pv = ps.tile([128, 512], FP32, tag="ps", name=f"psv{q}{tj}{g}_{sb}")
                    for ci, (c0, cs) in enumerate(C_CHUNKS):
                        nc.tensor.matmul(
                            pv[:ts_, :ns], xnt[ci][:cs, q * N + t0 : q * N + t0 + ts_],
                            wv[ci][:cs, n0 : n0 + ns],
                            start=(ci == 0), stop=(ci == NCC - 1),
                        )
                    if has_vbias:
                        nc.vector.tensor_add(
                            v[:ts_, :ns], pv[:ts_, :ns], vb[:ts_, n0 : n0 + ns]
                        )
                    else:
                        nc.scalar.copy(v[:ts_, :ns], pv[:ts_, :ns])
                    vt[(q, tj)] = v
            vgroups[g] = vt

        def stage_a(h, sb=sb, qkt=qkt, estate=estate, vgroups=vgroups,
                    stage_v=stage_v):
            # scores + exp for head h. S.T in [key, query] layout; queries
            # streamed through a 256-wide window of the packed 392 (f32r
            # needs >=256 free cols for full rate); out-of-batch slack
            # columns are ignored. Two key-chunks pack into one PSUM bank.
            #   batch A: window 0:256,   valid 0:196
            #   batch B: window 136:392, valid 60:256
            if h // 4 not in vgroups:
                stage_v(h // 4)
            qt = qkt[h // 3]
            kt = qkt[NG + h // 3]
            r0 = 32 * (h % 3)
            st = []
            for q in range(SB):
                w0 = 0 if q == 0 else W - 256
                v0 = 0 if q == 0 else 60
                s = ps.tile([128, 512], FP32, tag="ps", name=f"st{q}h{h}_{sb}")
                for tj, (t0, ts_) in enumerate(TOK_CHUNKS):
                    # seed the valid window with the bias (PE identity matmul,
                    # bf16), then accumulate the scores on top. Garbage slack
                    # columns have has_written unset, so the score matmul
                    # plain-writes them.
                    nc.tensor.matmul(
                        s[:ts_, tj * 256 + v0 : tj * 256 + v0 + N],
                        ident_b[:ts_, :ts_], biasT[h][:ts_, tj * N : tj * N + N],
                        start=True, stop=False, skip_group_check=True,
                    )
                    nc.tensor.matmul(
                        s[:ts_, tj * 256 : tj * 256 + 256],
                        kt[r0 : r0 + 32, q * N + t0 : q * N + t0 + ts_],
                        qt[r0 : r0 + 32, w0 : w0 + 256],
                        start=False, stop=True, skip_group_check=True,
                    )
                st.append(s)
            # E = exp(S.T + bias.T), blocks [A0|A1|B0|B1] at 196-stride; one
            # exp per batch straight from the PSUM windows.
            e = epool.tile([128, EW], F32R, tag="e", name=f"e{h}_{sb}")
            for q in range(SB):
                v0 = 0 if q == 0 else 60
                st3 = st[q][:, :].rearrange("p (c n) -> p c n", c=2)[:, :, v0 : v0 + N]
                e3 = e[:, q * W : (q + 1) * W].rearrange("p (c n) -> p c n", c=2)
                nc.scalar.activation(e3, st3, mybir.ActivationFunctionType.Exp)
            estate[h] = e

        def stage_b1(h, sb=sb, estate=estate, bstate=bstate, vgroups=vgroups):
            # denominator + reciprocal, and PV over unnormalized E
            e = estate[h]
            g = h // 4
            vt = vgroups[g]
            n0, ns = V_CHUNKS[g]
            hcol = h * DV - n0
            dn = ps.tile([1, W], FP32, tag="ps", name=f"dn{h}_{sb}")
            for tj, (t0, ts_) in enumerate(TOK_CHUNKS):
                nc.tensor.matmul(
                    dn[:1, :W], onescol[:ts_, :], e_pair(e, tj, ts_),
                    start=(tj == 0), stop=(tj == 1),
                )
            rc = rcpool.tile([1, W], F32R, tag="rc", name=f"rc{h}_{sb}")
            nc.vector.reciprocal(rc[:], dn[:1, :W])
            # 256-wide valid-at-start windows; both batches in one PSUM bank
            ou = ps.tile([128, 512], FP32, tag="ps", name=f"ou{h}_{sb}")
            for q in range(SB):
                for tj, (t0, ts_) in enumerate(TOK_CHUNKS):
                    w0 = q * W + tj * N
                    nc.tensor.matmul(
                        ou[:DV, q * 256 : q * 256 + 256],
                        vt[(q, tj)][:ts_, hcol : hcol + DV],
                        e[:ts_, w0 : w0 + 256],
                        start=(tj == 0), stop=(tj == 1),
                    )
            bstate[h] = (rc, ou)
            estate.pop(h)

        def stage_b2(h, sb=sb, od=od, bstate=bstate):
            # broadcast reciprocal, normalize O.T, ship to DRAM scratch
            rc, ou = bstate.pop(h)
            bc = ps.tile([128, W], FP32, tag="ps", name=f"bc{h}_{sb}")
            nc.tensor.matmul(
                bc[:, :W], onesrow[:1, :], rc[:1, :W], start=True, stop=True
            )
            rb = rbpool.tile([128, W], FP32, tag="rb", name=f"rb{h}_{sb}")
            nc.vector.tensor_copy(rb[:], bc[:, :W])
            onorm = onpool.tile([128, W], F32R, tag="onorm", name=f"on{h}_{sb}")
            nc.vector.tensor_mul(
                onorm[:, :].rearrange("p (b n) -> p b n", b=2, n=N),
                ou[:DV, :].rearrange("p (b n) -> p b n", b=2, n=256)[:, :, 0:N],
                rb[:, :].rearrange("p (b n) -> p b n", b=2, n=N),
            )
            nc.sync.dma_start(od[h], onorm[:DV, :])

        stage_a(0)
        stage_a(1)
        stage_b1(0)
        for h in range(H):
            if pending_finals and h in (0, 1, 2, 4):
                pending_finals.pop(0)()
            if h + 2 < H:
                stage_a(h + 2)
            if h + 1 < H:
                stage_b1(h + 1)
            stage_b2(h)
            if h in (3, 7, 11, 15) and sb + 1 < NSB:
                # hoist next superbatch's LayerNorm, one chunk at a time, so
                # its DVE/ACT work spreads under this superbatch's attention
                k = (3, 7, 11, 15).index(h)
                ln_chunk(sb + 1, k // 2, k % 2)

        # ---- proj: stream O.T back, accumulate heads (single pass) ----
        ft = ftpool.tile([128, NCC * W], FP32, tag="ft", name=f"ft{sb}")
        pp = {
            m: ps.tile([128, W], FP32, tag="ps", name=f"pp{m}_{sb}")
            for m in range(NCC)
        }
        for h in range(H):
            ots = otspool.tile([DV, W], F32R, tag="ots", name=f"ots{h}_{sb}")
            nc.sync.dma_start(ots[:, :], od[h])
            for m in range(NCC):
                c0, mc = C_CHUNKS[m]
                nc.tensor.matmul(
                    pp[m][:mc, :W], pw[h][:, c0 : c0 + mc], ots[:DV, :W],
                    start=(h == 0), stop=(h == H - 1),
                )
        for m in range(NCC):
            c0, mc = C_CHUNKS[m]
            nc.scalar.activation(
                ft[:mc, m * W : m * W + W], pp[m][:mc, :W],
                mybir.ActivationFunctionType.Identity, bias=pb[:mc, m : m + 1],
            )

        # ---- transpose back to token layout and store: deferred into the
        # next superbatch's head loop so the DMA-gated copies hide under
        # attention (flushed immediately on the last superbatch) ----
        def make_final(q, tj, ft=ft, sb=sb):
            def emit():
                b = sb * SB + q
                t0, ts_ = TOK_CHUNKS[tj]
                f = fpool.tile([128, C], FP32, tag="f", name=f"f{q}{tj}_{sb}")
                for m, (c0, mc) in enumerate(C_CHUNKS):
                    pt = ps.tile([128, 512], FP32, tag="ps", name=f"psf{m}{q}{tj}_{sb}")
                    src0 = m * W + q * N + t0
                    nc.tensor.transpose(
                        pt[:ts_, :mc], ft[:mc, src0 : src0 + ts_], ident[:mc, :mc]
                    )
                    if m % 2 == 0:
                        nc.scalar.copy(f[:ts_, c0 : c0 + mc], pt[:ts_, :mc])
                    else:
                        nc.vector.tensor_copy(f[:ts_, c0 : c0 + mc], pt[:ts_, :mc])
                nc.sync.dma_start(out_d[b, t0 : t0 + ts_, :], f[:ts_, :])
            return emit

        chunks = [make_final(q, tj) for q in range(SB) for tj in range(2)]
        if sb + 1 < NSB:
            pending_finals.extend(chunks)
        else:
            for c in chunks:
                c()


def _build(has_vbias: bool):
    nc = bass.Bass(
        "TRN2", target_bir_lowering=False, debug=False,
        enable_asserts=False, num_devices=NCORES,
    )
    aps = {}
    aps["x"] = nc.dram_tensor("x", [BPC, N, C], FP32, kind="ExternalInput").ap()
    aps["wqkt"] = nc.dram_tensor("wqkt", [C, 2 * C], F32R, kind="ExternalInput").ap()
    aps["wvt"] = nc.dram_tensor("wvt", [C, DH], F32R, kind="ExternalInput").ap()
    aps["pwt"] = nc.dram_tensor("pwt", [DH, C], F32R, kind="ExternalInput").ap()
    aps["biasT"] = nc.dram_tensor("biasT", [H, N, N], BF16, kind="ExternalInput").ap()
    aps["qkb"] = nc.dram_tensor("qkb", [128, 2 * NG], FP32, kind="ExternalInput").ap()
    aps["pb"] = nc.dram_tensor("pb", [128, NCC], FP32, kind="ExternalInput").ap()
    if has_vbias:
        aps["vb"] = nc.dram_tensor("vb", [128, DH], FP32, kind="ExternalInput").ap()
    aps["out"] = nc.dram_tensor("out", [BPC, N, C], FP32, kind="ExternalOutput").ap()

    with tile.TileContext(nc) as tc, ExitStack() as ctx:
        with nc.allow_low_precision(reason="f32r matmul pipeline"):
            _emit(ctx, tc, aps, has_vbias)
    _split_multiwaits(nc)
    return nc


_BUILD_CACHE: dict = {}


def _prep_host(x, ln_g, ln_b, qkv_w, qkv_b, proj_w, proj_b, attn_biases, bias_idxs):
    """Permute/fold weights host-side. Returns (in_map_consts, has_vbias)."""
    f32 = np.float32
    qkv_w = np.asarray(qkv_w, f32)
    qkv_b = np.asarray(qkv_b, f32)
    ln_g = np.asarray(ln_g, f32)
    ln_b = np.asarray(ln_b, f32)
    proj_w = np.asarray(proj_w, f32)
    proj_b = np.asarray(proj_b, f32)
    attn_biases = np.asarray(attn_biases, f32)
    bias_idxs = np.asarray(bias_idxs)

    per = 2 * KD + DV  # 192 rows per head in qkv_w
    wq = np.concatenate([qkv_w[h * per : h * per + KD] for h in range(H)], 0)
    wk = np.concatenate([qkv_w[h * per + KD : h * per + 2 * KD] for h in range(H)], 0)
    wv = np.concatenate([qkv_w[h * per + 2 * KD : (h + 1) * per] for h in range(H)], 0)
    bq = np.concatenate([qkv_b[h * per : h * per + KD] for h in range(H)], 0)
    bk = np.concatenate([qkv_b[h * per + KD : h * per + 2 * KD] for h in range(H)], 0)
    bv = np.concatenate([qkv_b[h * per + 2 * KD : (h + 1) * per] for h in range(H)], 0)

    # fold LN affine: xn = xn0 * g + beta  =>  W_eff = W*g, b_eff = W@beta + b
    wq_eff = (wq * ln_g[None, :] * SCALE).astype(f32)
    wk_eff = (wk * ln_g[None, :]).astype(f32)
    wv_eff = (wv * ln_g[None, :]).astype(f32)
    bq_eff = ((wq @ ln_b + bq) * SCALE).astype(f32)
    bk_eff = (wk @ ln_b + bk).astype(f32)
    bv_eff = (wv @ ln_b + bv).astype(f32)

    wqkt = np.concatenate([wq_eff.T, wk_eff.T], axis=1).copy()  # [576, 1152]
    wvt = wv_eff.T.copy()                                        # [576, 2304]
    pwt = proj_w.T.copy()                                        # [2304, 576]

    qkb = np.zeros((128, 2 * NG), f32)
    for j in range(2 * NG):
        src = bq_eff if j < NG else bk_eff
        g = j % NG
        qkb[:GROWS, j] = src[g * GROWS : (g + 1) * GROWS]
    pb = np.zeros((128, NCC), f32)
    for m, (c0, mc) in enumerate(C_CHUNKS):
        pb[:mc, m] = proj_b[c0 : c0 + mc]

    # per-head relative-position bias, gathered and transposed to [key, query]
    biasT = attn_biases[:, bias_idxs.T].astype(ml_dtypes.bfloat16)  # [H, N, N]

    has_vbias = bool(np.any(bv_eff != 0.0))
    consts = {
        "wqkt": wqkt, "wvt": wvt, "pwt": pwt,
        "biasT": np.ascontiguousarray(biasT),
        "qkb": qkb, "pb": pb,
    }
    if has_vbias:
        consts["vb"] = np.broadcast_to(bv_eff[None, :], (128, DH)).copy()
    return consts, has_vbias


def kernel(**inputs) -> np.ndarray:
    x = np.asarray(inputs["x"], np.float32)
    consts, has_vbias = _prep_host(
        x, inputs["ln_g"], inputs["ln_b"], inputs["qkv_w"], inputs["qkv_b"],
        inputs["proj_w"], inputs["proj_b"], inputs["attn_biases"],
        inputs["bias_idxs"],
    )
    key = has_vbias
    if key not in _BUILD_CACHE:
        _BUILD_CACHE[key] = _build(has_vbias)
    nc = _BUILD_CACHE[key]

    in_maps = []
    for c in range(NCORES):
        m = {"x": np.ascontiguousarray(x[c * BPC : (c + 1) * BPC])}
        m.update(consts)
        in_maps.append(m)
    res = run_bass_kernel_spmd(nc, in_maps, list(range(NCORES)))
    out = np.concatenate([res.results[c]["out"] for c in range(NCORES)], axis=0)
    return out.astype(np.float32)

